# revision 4
# baseline (speedup 1.0000x reference)
"""CBConv2d (change-based conv) Trainium2 kernel, 8-core SPMD.

Reference semantics (B=1, C=64, H=W=512, 3x3 SAME conv):
  changed = any_c(|inp - prev_input| > 0.1)            # [H, W]
  dilated = maxpool3x3(changed)                        # [H, W]
  out     = dilated ? (conv2d(inp, w) + bias) : prev_output

Sharding: H split across 8 cores (64 rows each), halos materialized on host.

Per-core device pipeline (4 tiles of 16 output rows):
  - inputs bf16 (host pre-cast); prev_output and out are ALSO bf16 on the
    wire (upcast to fp32 on host) -- tolerance is 2e-2, bf16 adds ~4e-3.
  - conv runs on TensorE in 64x64 array-tiled mode: 4 concurrent K=64
    matmuls in the 4 array quadrants (T0/T2/T8/T10), one output row each,
    rows paired (s, s+4) within each 8-row half -> PSUM banks hold
    [row j | row j+4] across the partition halves. This doubles PE
    throughput vs the 128-contraction block-diagonal scheme (no wasted
    zero quadrants).
  - change mask: DVE subtract, then ONE fused DVE tensor_scalar
    (abs_max(d,0) then is_gt thr) -> 0/1 indicator; per-pixel change
    count AND the H-dilation come from 10 matmuls with 3-wide banded
    ones weights (cnt rows permuted to pair-block order); W-dilation is
    2 DVE adds; PE ones-matmuls broadcast the dilated count across
    partitions into PSUM; one copy_predicated per pair merges conv over
    prev_output.

Mask exactness note: inputs are bf16-rounded, so pixels whose |diff| sits
within ~0.4% of the threshold can flip vs the fp32 reference. A flipped
pixel only affects the output if its entire 3x3 neighborhood has no other
changed pixel; with this data distribution (~95% changed) the expected
number of affected output pixels is ~1e-7.
"""
import numpy as np
import ml_dtypes

import concourse.bass as bass
import concourse.mybir as mybir
import concourse.tile as tile
from concourse import bacc
from concourse.bass_utils import run_bass_kernel_spmd

F32 = mybir.dt.float32
BF16 = mybir.dt.bfloat16
BF = ml_dtypes.bfloat16

C = 64          # channels
H = W = 512     # spatial
NCORES = 8
RPC = H // NCORES          # rows per core (64)
R = 16                     # output rows per tile
NT = RPC // R              # tiles per core (4)
NPAD = R + 2               # padded rows per tile (18)
G = 10                     # rows per partition-group (overlapping: lower=0..9, upper=8..17)
WP = W + 2                 # padded width (514)
THR = float(np.float32(0.1))

# pair-block structure: block b of the [128, 8*W] pout/out tiles holds
# out row LROW[b] on partitions 0:64 and UROW[b] on partitions 64:128.
LROW = [0, 1, 2, 3, 8, 9, 10, 11]
UROW = [4, 5, 6, 7, 12, 13, 14, 15]
# cnt/dil row u corresponds to out row ROWPERM[u] (so dil1 = natural reshape)
ROWPERM = LROW + UROW

_cached = {}


def build_nc(loop_iters: int = 0, variant: str = "full"):
    """Build the per-core Bass program. loop_iters>0 wraps the whole pipeline
    in a For_i loop that re-executes it (for slope-based timing).

    variant tokens (comma-joined):
      indact  - indicator via ACT Square + Relu(bias) instead of fused DVE TS
      indsplit- ACT Square + DVE is_gt
      evdve   - do half the conv evacuations on DVE (bias via const tile)
      nosel   - plain copy instead of copy_predicated
      nomb    - also skip mask-broadcast matmuls
      nodil   - also skip W-dilation + dil1 DMA
      nocnt   - also skip count matmuls
      noind   - also skip indicator ops (pure conv kernel)
      noconv  - skip conv matmuls + evac (mask pipeline only; copy prev->out)
    """
    has_ind = "noind" not in variant
    has_cnt = has_ind and "nocnt" not in variant
    has_dil = has_cnt and "nodil" not in variant
    has_mb = has_dil and "nomb" not in variant
    has_sel = has_mb and "nosel" not in variant
    has_conv = "noconv" not in variant
    ind_act = "indact" in variant
    ind_split = "indsplit" in variant
    ev_dve = "evdve" in variant

    nc = bacc.Bacc("TRN2", target_bir_lowering=False, debug=False,
                   enable_asserts=True, num_devices=NCORES)

    xin = nc.dram_tensor("xin", [NT, 128, G * WP], BF16, kind="ExternalInput")
    pin = nc.dram_tensor("pin", [NT, 128, G * WP], BF16, kind="ExternalInput")
    pout = nc.dram_tensor("pout", [NT, 128, 8 * W], BF16, kind="ExternalInput")
    wt = nc.dram_tensor("wt", [128, 9 * 64], BF16, kind="ExternalInput")
    sel = nc.dram_tensor("sel", [128, G * R], BF16, kind="ExternalInput")
    sel2x = nc.dram_tensor("sel2x", [2, 128], BF16, kind="ExternalInput")
    biasv = nc.dram_tensor("biasv", [128, 1], F32, kind="ExternalInput")
    outd = nc.dram_tensor("out", [NT, 128, 8 * W], BF16, kind="ExternalOutput")

    with tile.TileContext(nc) as tc:
        with tc.tile_pool(name="consts", bufs=1) as cpool, \
             tc.tile_pool(name="io", bufs=2) as iopool, \
             tc.tile_pool(name="mask", bufs=2) as mpool, \
             tc.tile_pool(name="cnt", bufs=2, space="PSUM") as cntpool, \
             tc.tile_pool(name="conv", bufs=4, space="PSUM") as convpool, \
             tc.tile_pool(name="mb", bufs=2, space="PSUM") as mbpool:

            wtt = cpool.tile([128, 9 * 64], BF16)
            selt = cpool.tile([128, G * R], BF16)
            sel2xt = cpool.tile([2, 128], BF16)
            biast = cpool.tile([128, 1], F32)
            negthr2 = cpool.tile([128, 1], F32)
            bias_bc = cpool.tile([128, W], F32)
            hs = cpool.tile([R, WP], F32)       # persistent zero-padded edges
            nc.sync.dma_start(out=wtt[:], in_=wt[:])
            nc.sync.dma_start(out=selt[:], in_=sel[:])
            nc.sync.dma_start(out=sel2xt[:], in_=sel2x[:])
            nc.sync.dma_start(out=biast[:], in_=biasv[:])
            nc.vector.memset(negthr2[:], -(THR * THR))
            nc.vector.memset(hs[:], 0.0)
            if ev_dve:
                # bias broadcast along W for DVE-side evacuation
                nc.vector.memset(bias_bc[:], 0.0)
                nc.scalar.activation(bias_bc[:], bias_bc[:],
                                     mybir.ActivationFunctionType.Identity,
                                     bias=biast[:])

            def emit_tile(t):
                xt = iopool.tile([128, G * WP], BF16, tag="xt")
                pt = iopool.tile([128, G * WP], BF16, tag="pt")
                pvt = iopool.tile([128, 8 * W], BF16, tag="pvt")
                nc.sync.dma_start(out=xt[:], in_=xin[t])
                nc.sync.dma_start(out=pt[:], in_=pin[t])
                nc.sync.dma_start(out=pvt[:], in_=pout[t])

                dil1 = None
                if has_ind:
                    # --- change indicator ---
                    ind = mpool.tile([128, G * WP], BF16, tag="ind")
                    nc.vector.tensor_tensor(out=ind[:], in0=xt[:], in1=pt[:],
                                            op=mybir.AluOpType.subtract)
                    if ind_act:
                        nc.scalar.activation(ind[:], ind[:],
                                             mybir.ActivationFunctionType.Square)
                        nc.scalar.activation(ind[:], ind[:],
                                             mybir.ActivationFunctionType.Relu,
                                             bias=negthr2[:])
                    else:
                        # split: ACT Square, DVE threshold -> 0/1
                        nc.scalar.activation(ind[:], ind[:],
                                             mybir.ActivationFunctionType.Square)
                        nc.vector.tensor_scalar(out=ind[:], in0=ind[:],
                                                scalar1=THR * THR, scalar2=None,
                                                op0=mybir.AluOpType.is_gt)

                if has_cnt:
                    # --- change count + H-dilation via banded matmuls ---
                    # cnt row u = out row ROWPERM[u]; one 128-deep MM per
                    # k-slot contracts group0 row k AND group1 row k+8 (rows
                    # 8,9 counted twice -- harmless, only nonzero-ness used).
                    cnt = cntpool.tile([R, W], F32, tag="cnt")
                    for k in range(G):
                        nc.tensor.matmul(
                            cnt[:],
                            selt[:, k * R:(k + 1) * R],
                            ind[:, k * WP + 1:k * WP + 1 + W],
                            start=(k == 0), stop=(k == G - 1))

                if has_dil:
                    # --- W-dilation on [R, W+2] (hs edges stay zero) ---
                    nc.vector.tensor_copy(out=hs[:, 1:W + 1], in_=cnt[:])
                    t1 = mpool.tile([R, W + 1], F32, tag="t1")
                    nc.vector.tensor_tensor(out=t1[:], in0=hs[:, 0:W + 1],
                                            in1=hs[:, 1:WP],
                                            op=mybir.AluOpType.add)
                    dil = mpool.tile([R, W], BF16, tag="dil")
                    nc.vector.tensor_tensor(out=dil[:], in0=t1[:, 0:W],
                                            in1=hs[:, 2:WP],
                                            op=mybir.AluOpType.add)
                    dil1 = mpool.tile([2, 8 * W], BF16, tag="dil1")
                    nc.scalar.dma_start(out=dil1[:], in_=dil[:])

                conv_sb = iopool.tile([128, 8 * W], BF16, tag="conv_sb")
                if has_conv:
                    # --- conv, 64x64 array-tiled: 4 concurrent K=64 MMs ---
                    # quadrants: T0=(r0:64,c0:64) row s     T2=(r0:64,c64:)   row s+4
                    #            T8=(r64:,c0:64)  row 8+s   T10=(r64:,c64:)   row 12+s
                    # cbA = block s = rows (s, s+4) from G0; cbB = block 4+s
                    # = rows (8+s, 12+s) from G1 (G1-local s, s+4).
                    for s in range(4):
                        cbA = convpool.tile([128, W], F32, tag="cb")
                        cbB = convpool.tile([128, W], F32, tag="cb")
                        taps = [(dh, dw) for dh in range(3) for dw in range(3)]
                        for i, (dh, dw) in enumerate(taps):
                            ti = dh * 3 + dw
                            st, sp = (i == 0), (i == len(taps) - 1)
                            wlo = wtt[0:64, ti * 64:(ti + 1) * 64]
                            whi = wtt[64:128, ti * 64:(ti + 1) * 64]
                            nc.tensor.matmul(
                                cbA[0:64], wlo,
                                xt[0:64, (s + dh) * WP + dw:
                                   (s + dh) * WP + dw + W],
                                start=st, stop=sp)
                            nc.tensor.matmul(
                                cbA[64:128], wlo,
                                xt[0:64, (s + 4 + dh) * WP + dw:
                                   (s + 4 + dh) * WP + dw + W],
                                start=st, stop=sp)
                            nc.tensor.matmul(
                                cbB[0:64], whi,
                                xt[64:128, (s + dh) * WP + dw:
                                   (s + dh) * WP + dw + W],
                                start=st, stop=sp)
                            nc.tensor.matmul(
                                cbB[64:128], whi,
                                xt[64:128, (s + 4 + dh) * WP + dw:
                                   (s + 4 + dh) * WP + dw + W],
                                start=st, stop=sp)
                        # --- evacuate conv + bias (blocks s and 4+s) ---
                        for cb, b in ((cbA, s), (cbB, 4 + s)):
                            sl = slice(b * W, (b + 1) * W)
                            if ev_dve and b >= 4:
                                nc.vector.tensor_tensor(
                                    out=conv_sb[:, sl], in0=cb[:],
                                    in1=bias_bc[:],
                                    op=mybir.AluOpType.add)
                            else:
                                nc.scalar.activation(
                                    conv_sb[:, sl], cb[:],
                                    mybir.ActivationFunctionType.Identity,
                                    bias=biast[:])

                for b in range(8):
                    sl = slice(b * W, (b + 1) * W)
                    if has_mb:
                        # --- broadcast dilated counts for block b ---
                        mb = mbpool.tile([128, W], F32, tag="mb")
                        nc.tensor.matmul(mb[:], sel2xt[:],
                                         dil1[:, b * W:(b + 1) * W],
                                         start=True, stop=True)

                    # --- merge conv over prev_output ---
                    if has_sel and has_conv:
                        nc.vector.copy_predicated(
                            pvt[:, sl], mb[:].bitcast(mybir.dt.int32),
                            conv_sb[:, sl])
                    elif has_conv:
                        nc.vector.tensor_copy(out=pvt[:, sl],
                                              in_=conv_sb[:, sl])

                nc.scalar.dma_start(out=outd[t], in_=pvt[:])

            if loop_iters > 0:
                with tc.For_i(0, loop_iters, 1,
                              hint_engines=(mybir.EngineType.PE,
                                            mybir.EngineType.DVE,
                                            mybir.EngineType.Activation,
                                            mybir.EngineType.SP)):
                    for t in range(NT):
                        emit_tile(t)
            else:
                for t in range(NT):
                    emit_tile(t)

    nc.compile()
    return nc


def host_prep(inp, prev_input, prev_output, weight, bias):
    """Build per-core in_maps."""
    inp = np.asarray(inp)
    prev_input = np.asarray(prev_input)
    prev_output = np.asarray(prev_output)
    weight = np.asarray(weight)
    bias = np.asarray(bias)

    xpad = np.zeros((C, H + 2, WP), dtype=BF)
    ppad = np.zeros((C, H + 2, WP), dtype=BF)
    xpad[:, 1:H + 1, 1:W + 1] = inp[0].astype(BF)
    ppad[:, 1:H + 1, 1:W + 1] = prev_input[0].astype(BF)

    # weights: wt[ci + 64g, (dh*3+dw)*64 + co] = weight[co, ci, dh, dw]
    wtap = weight.transpose(1, 2, 3, 0).reshape(C, 9 * C).astype(BF)
    wt = np.concatenate([wtap, wtap], axis=0)  # [128, 576]

    # sel bands, cnt rows permuted: cnt row u <-> out row ROWPERM[u].
    # group0 handles padded rows p=0..9 (covers out rows 0..7), group1
    # p=8..17 (out rows 8..15): sel[*, k*R + u] = 1 iff p-2 <= ROWPERM[u] <= p.
    selA = np.zeros((G, R), dtype=BF)
    selB = np.zeros((G, R), dtype=BF)
    for u in range(R):
        rr = ROWPERM[u]
        for p in range(rr, rr + 3):        # padded rows rr..rr+2
            if rr <= 7:
                selA[p, u] = 1
            else:
                selB[p - 8, u] = 1
    sel = np.empty((128, G * R), dtype=BF)
    sel[:64] = selA.reshape(1, G * R)
    sel[64:] = selB.reshape(1, G * R)

    sel2x = np.zeros((2, 128), dtype=BF)
    sel2x[0, :64] = 1
    sel2x[1, 64:] = 1
    biasv = np.tile(bias.astype(np.float32).reshape(-1, 1), (2, 1))  # [128,1]

    lrow = np.array(LROW)
    urow = np.array(UROW)

    in_maps = []
    for c in range(NCORES):
        r0 = c * RPC

        def slab(pad):
            s = np.empty((NT, 128, G * WP), dtype=BF)
            for t in range(NT):
                rows = pad[:, r0 + 16 * t: r0 + 16 * t + NPAD, :]  # [C,18,WP]
                s[t, :64] = rows[:, 0:10].reshape(C, G * WP)
                s[t, 64:] = rows[:, 8:18].reshape(C, G * WP)
            return s

        po = prev_output[0][:, r0:r0 + RPC, :].astype(BF)  # [C, 64, W]
        po = po.reshape(C, NT, R, W)
        pot = np.empty((NT, 128, 8 * W), dtype=BF)
        for t in range(NT):
            pot[t, :64] = po[:, t, lrow].transpose(0, 1, 2).reshape(C, 8 * W)
            pot[t, 64:] = po[:, t, urow].reshape(C, 8 * W)

        in_maps.append({
            "xin": slab(xpad), "pin": slab(ppad), "pout": pot,
            "wt": wt, "sel": sel, "sel2x": sel2x, "biasv": biasv,
        })
    return in_maps


def host_post(results):
    """Reassemble [NCORES] x [NT, 128, 8*W] bf16 -> [1, C, H, W] fp32."""
    out = np.empty((1, C, H, W), dtype=np.float32)
    lrow = np.array(LROW)
    urow = np.array(UROW)
    for c, res in enumerate(results):
        o = res["out"].reshape(NT, 2, C, 8, W).astype(np.float32)
        blk = np.empty((NT, C, R, W), dtype=np.float32)
        blk[:, :, lrow] = o[:, 0].transpose(0, 1, 2, 3)
        blk[:, :, urow] = o[:, 1]
        out[0, :, c * RPC:(c + 1) * RPC, :] = \
            blk.transpose(1, 0, 2, 3).reshape(C, RPC, W)
    return out


def kernel(inp, prev_input, prev_output, weight, bias):
    if "nc" not in _cached:
        _cached["nc"] = build_nc(0)
    nc = _cached["nc"]
    in_maps = host_prep(inp, prev_input, prev_output, weight, bias)
    res = run_bass_kernel_spmd(nc, in_maps, core_ids=list(range(NCORES)))
    return host_post(res.results)


if __name__ == "__main__":
    rng = np.random.default_rng(0)
    inp = rng.standard_normal((1, C, H, W), dtype=np.float32)
    prev_input = inp + 0.05 * rng.standard_normal((1, C, H, W), dtype=np.float32)
    prev_output = rng.standard_normal((1, C, H, W), dtype=np.float32)
    weight = (0.05 * rng.standard_normal((C, C, 3, 3))).astype(np.float32)
    bias = rng.standard_normal(C).astype(np.float32)
    out = kernel(inp=inp, prev_input=prev_input, prev_output=prev_output,
                 weight=weight, bias=bias)
    print("out", out.shape, out.dtype, float(np.abs(out).mean()))
